# revision 1
# baseline (speedup 1.0000x reference)
"""Distributed causal attention head for TRN2 (8 NeuronCores).

Problem: B=4, S=4096, D=1024, H=64 fp32.
  q,k,v = x @ W{q,k,v}; scores = q k^T / sqrt(H); causal softmax; out = P v.

Sharding (fully SPMD-uniform, one NEFF for all 8 cores):
  - 4 batches x 2 cores per batch (pair replica groups [[0,1],[2,3],[4,5],[6,7]]).
  - Within a pair, the KEY dimension is split by interleaved 128-row chunks:
    core g owns global key chunks {2i+g}. Each core computes K^T/V (and its
    share of Q^T) from only its own 2048 input rows.
  - Q^T is pair-AllGathered (every core needs all 4096 query columns).
  - Both cores process ALL 8 query blocks of 512; for q-block t each core has
    exactly 2(t+1) local causal key chunks -> identical instruction sequence
    on every core. Causal masking inside the two diagonal chunks uses
    per-core 0/1 mask inputs (multiplicative, after exp).
  - Per-core partial (numerator | denominator) = [4096, 65] fp32 is merged
    with a pair ReduceScatter(add); core g keeps rows [2048g, 2048(g+1)).
    Host divides num/den and re-assembles.

Compute layout notes:
  - All matmuls contract on the partition dim. Scores are computed
    transposed: S_T[k, q] = matmul(lhsT=K^T[:, kchunk], rhs=Q^T[:, qblock]).
  - exp runs on the scalar engine straight out of PSUM (scale=1/8 fused).
  - V is augmented with a ones column, so the AV matmul also produces the
    softmax denominator for free (row 64 of the [65, 512] accumulator).
  - bf16 operands everywhere on the PE (f32 PSUM accumulation); input is
    cast to bf16 on the host, and x^T is loaded via DMA transpose.
"""

import sys

sys.path.insert(0, "/opt/trn_rl_repo")

import numpy as np
import ml_dtypes

B, S, D, H = 4, 4096, 1024, 64
RPC = S // 2            # rows (keys/queries) owned per core
QB = 512                # query block width
NQB = S // QB           # 8 query blocks
NKC = RPC // 128        # 16 local key chunks
NSG = RPC // QB         # 4 own-row groups for projection
BF16 = ml_dtypes.bfloat16
PAIRS = [[0, 1], [2, 3], [4, 5], [6, 7]]

_CACHE = {}


def _build():
    import concourse.bass as bass
    import concourse.mybir as mybir
    from concourse import bacc, tile
    from concourse.masks import make_identity
    from concourse.bass import ts

    f32 = mybir.dt.float32
    bf16 = mybir.dt.bfloat16
    Alu = mybir.AluOpType
    Act = mybir.ActivationFunctionType

    nc = bacc.Bacc(None, target_bir_lowering=False)

    x_ext = nc.declare_dram_parameter("x", [RPC, D], bf16, isOutput=False)
    wqk_ext = nc.declare_dram_parameter("wqk", [D, 128], bf16, isOutput=False)
    wv_ext = nc.declare_dram_parameter("wv", [D, H], bf16, isOutput=False)
    mask_ext = nc.declare_dram_parameter("mask", [2, 128, QB], bf16, isOutput=False)
    out_ext = nc.declare_dram_parameter("out", [RPC, H + 1], f32, isOutput=True)

    with tile.TileContext(nc) as tc:
        with (
            tc.tile_pool(name="persist", bufs=1) as persist,
            tc.tile_pool(name="dram", bufs=1, space="DRAM") as dram,
        ):
            # --- persistent SBUF tensors ---
            wqk_sb = persist.tile([128, 8, 128], bf16, tag="wqk")
            wv_sb = persist.tile([128, 8, H], bf16, tag="wv")
            mask_sb = persist.tile([128, 2, QB], bf16, tag="mask")
            qT_own = persist.tile([64, RPC], bf16, tag="qT_own")
            qT_full = persist.tile([64, S], bf16, tag="qT_full")
            kT = persist.tile([64, RPC], bf16, tag="kT")
            vT = persist.tile([64, RPC], bf16, tag="vT")
            v_all = persist.tile([128, NKC, H + 1], bf16, tag="v_all")
            id_bf = persist.tile([128, 128], bf16, tag="id_bf")
            id_f32 = persist.tile([128, 128], f32, tag="id_f32")

            make_identity(nc, id_bf.opt())
            make_identity(nc, id_f32.opt())
            nc.vector.memset(v_all[:, :, H], 1.0)

            for dc in range(8):
                nc.sync.dma_start(out=wqk_sb[:, dc, :], in_=wqk_ext[ts(dc, 128), :])
                nc.sync.dma_start(out=wv_sb[:, dc, :], in_=wv_ext[ts(dc, 128), :])
            nc.sync.dma_start(out=mask_sb[:, 0, :], in_=mask_ext[0])
            nc.sync.dma_start(out=mask_sb[:, 1, :], in_=mask_ext[1])

            # --- phase 1: projections over own rows ---
            with (
                tc.tile_pool(name="xt", bufs=2) as xt_pool,
                tc.tile_pool(name="pj", bufs=2, space="PSUM") as pj_pool,
                tc.tile_pool(name="pv", bufs=2, space="PSUM") as pv_pool,
            ):
                for sg in range(NSG):
                    xT = xt_pool.tile([128, 8, QB], bf16, tag="xT")
                    for dc in range(8):
                        nc.sync.dma_start(
                            out=xT[:, dc, :],
                            in_=x_ext[ts(sg, QB), ts(dc, 128)],
                            transpose=True,
                        )
                    qk_ps = pj_pool.tile([128, QB], f32, tag="qk")
                    for dc in range(8):
                        nc.tensor.matmul(
                            qk_ps[:],
                            lhsT=wqk_sb[:, dc, :],
                            rhs=xT[:, dc, :],
                            start=(dc == 0),
                            stop=(dc == 7),
                        )
                    nc.any.tensor_copy(qT_own[:, ts(sg, QB)], qk_ps[0:64, :])
                    nc.any.tensor_copy(kT[:, ts(sg, QB)], qk_ps[64:128, :])
                    v_ps = pv_pool.tile([64, QB], f32, tag="v")
                    for dc in range(8):
                        nc.tensor.matmul(
                            v_ps[:],
                            lhsT=wv_sb[:, dc, :],
                            rhs=xT[:, dc, :],
                            start=(dc == 0),
                            stop=(dc == 7),
                        )
                    nc.any.tensor_copy(vT[:, ts(sg, QB)], v_ps[:])

            # --- phase 2: Q^T pair AllGather; V^T -> V natural transposes ---
            q_bounce = dram.tile([64, RPC], bf16, tag="q_bounce")
            q_gath = dram.tile([2, 64, RPC], bf16, tag="q_gath")
            nc.sync.dma_start(out=q_bounce[:], in_=qT_own[:])
            nc.gpsimd.collective_compute(
                "AllGather",
                Alu.bypass,
                replica_groups=PAIRS,
                ins=[q_bounce.opt()],
                outs=[q_gath.opt()],
            )

            with tc.tile_pool(name="vt_ps", bufs=2, space="PSUM") as vt_ps_pool:
                for i in range(NKC):
                    tp = vt_ps_pool.tile([128, H], bf16, tag="tp")
                    nc.tensor.transpose(tp[:], vT[:, ts(i, 128)], id_bf[0:64, 0:64])
                    nc.any.tensor_copy(v_all[:, i, 0:H], tp[:])

            # gathered Q^T -> SBUF in global column order:
            # global chunk c lives at gathered slot c%2, local offset (c//2)*128
            for c in range(S // 128):
                nc.sync.dma_start(
                    out=qT_full[:, ts(c, 128)],
                    in_=q_gath[c % 2, :, ts(c // 2, 128)],
                )

            # --- phase 3: attention ---
            merge = dram.tile([S, H + 1], f32, tag="merge")
            with (
                tc.tile_pool(name="st", bufs=2, space="PSUM") as st_pool,
                tc.tile_pool(name="av", bufs=2, space="PSUM") as av_pool,
                tc.tile_pool(name="tr", bufs=2, space="PSUM") as tr_pool,
                tc.tile_pool(name="p", bufs=3) as p_pool,
                tc.tile_pool(name="o", bufs=2) as o_pool,
                tc.tile_pool(name="m", bufs=3) as m_pool,
            ):
                for t in range(NQB):
                    E = 2 * (t + 1)  # local causal chunk count
                    av_ps = av_pool.tile([H + 1, QB], f32, tag="av")
                    for i in range(E):
                        st_ps = st_pool.tile([128, QB], f32, tag="st")
                        nc.tensor.matmul(
                            st_ps[:],
                            lhsT=kT[:, ts(i, 128)],
                            rhs=qT_full[:, ts(t, QB)],
                            start=True,
                            stop=True,
                        )
                        p_sb = p_pool.tile([128, QB], bf16, tag="p")
                        nc.scalar.activation(p_sb[:], st_ps[:], Act.Exp, scale=0.125)
                        if i >= E - 2:
                            j = i - (E - 2)
                            nc.vector.tensor_tensor(
                                p_sb[:], p_sb[:], mask_sb[:, j, :], Alu.mult
                            )
                        nc.tensor.matmul(
                            av_ps[:],
                            lhsT=v_all[:, i, :],
                            rhs=p_sb[:],
                            start=(i == 0),
                            stop=(i == E - 1),
                        )
                    o_sb = o_pool.tile([H + 1, QB], f32, tag="o")
                    nc.any.tensor_copy(o_sb[:], av_ps[:])
                    for a in range(4):
                        tr_ps = tr_pool.tile([128, H + 1], f32, tag="tr")
                        nc.tensor.transpose(
                            tr_ps[:], o_sb[:, ts(a, 128)], id_f32[0 : H + 1, 0 : H + 1]
                        )
                        m_sb = m_pool.tile([128, H + 1], f32, tag="m")
                        nc.vector.tensor_copy(m_sb[:], tr_ps[:])
                        nc.sync.dma_start(
                            out=merge[t * QB + a * 128 : t * QB + (a + 1) * 128, :],
                            in_=m_sb[:],
                        )

            # --- phase 4: pair ReduceScatter + output ---
            rs_out = dram.tile([RPC, H + 1], f32, tag="rs_out")
            nc.gpsimd.collective_compute(
                "ReduceScatter",
                Alu.add,
                replica_groups=PAIRS,
                ins=[merge.opt()],
                outs=[rs_out.opt()],
            )
            nc.sync.dma_start(out=out_ext[:], in_=rs_out[:])

    nc.finalize()
    return nc


def _make_masks(g: int) -> np.ndarray:
    # mask[j][kk, qq] = 1 if query (512t + qq) >= key 128*(4t + 2j + g) + kk
    m = np.zeros((2, 128, QB), dtype=np.float32)
    for j in range(2):
        dk = 128 * (2 * j + g) + np.arange(128)[:, None]
        dq = np.arange(QB)[None, :]
        m[j] = (dq >= dk).astype(np.float32)
    return m.astype(BF16)


def _shard_inputs(input, Wq, Wk, Wv):
    x = np.ascontiguousarray(input)
    wqk = np.concatenate([Wq, Wk], axis=1).astype(BF16)
    wv = np.ascontiguousarray(Wv).astype(BF16)
    masks = [_make_masks(0), _make_masks(1)]
    in_maps = []
    for c in range(8):
        b, g = c // 2, c % 2
        xs = x[b].reshape(S // 128, 128, D)[g::2].reshape(RPC, D).astype(BF16)
        in_maps.append(
            {"x": np.ascontiguousarray(xs), "wqk": wqk, "wv": wv, "mask": masks[g]}
        )
    return in_maps


def _unshard(results):
    out = np.empty((B, S, H), dtype=np.float32)
    for b in range(B):
        merged = np.concatenate(
            [results[2 * b]["out"], results[2 * b + 1]["out"]], axis=0
        )
        out[b] = merged[:, :H] / merged[:, H : H + 1]
    return out


def _run(inputs, trace=False):
    from concourse.bass_utils import run_bass_kernel_spmd

    if "nc" not in _CACHE:
        _CACHE["nc"] = _build()
    nc = _CACHE["nc"]
    in_maps = _shard_inputs(**inputs)
    res = run_bass_kernel_spmd(nc, in_maps, core_ids=list(range(8)), trace=trace)
    out = _unshard(res.results)
    return out, res


def kernel(**inputs) -> np.ndarray:
    out, _ = _run(inputs, trace=False)
    return out



# revision 2
# speedup vs baseline: 1.7791x; 1.7791x over previous
"""Distributed causal attention head for TRN2 (8 NeuronCores), v2.

Problem: B=4, S=4096, D=1024, H=64 fp32.
  q,k,v = x @ W{q,k,v}; scores = q k^T / sqrt(H); causal softmax; out = P v.

Sharding (fully SPMD-uniform, one NEFF for all 8 cores):
  - 4 batches x 2 cores per batch (pair replica groups [[0,1],[2,3],[4,5],[6,7]]).
  - Within a pair, the KEY dimension is split by interleaved 128-row chunks:
    core g owns global key chunks {2i+g}. Host pre-transposes each core's
    2048 input rows to x^T [1024, 2048] bf16 so all DMA loads are linear.
  - Q^T is pair-AllGathered; both cores process all 8 global query blocks of
    512. For q-block t each core has 2(t+1) local causal key chunks.
  - Per-core partial (numerator | denominator) = [65, 4096] fp32 goes straight
    to DRAM; the HOST merges the pair (add), divides, and transposes. No
    ReduceScatter on device.

Compute layout:
  - Projections contract on the partition dim with x^T tiles as rhs.
    wkq = [Wk | Wq] packed 128-wide; V is computed in natural [keys, 64]
    layout via small matmuls with x^T chunks as the stationary operand
    (no PE transposes anywhere).
  - Scores are computed transposed, 2x PE row tiling (64-contract):
    tile (0,0) does local chunks 0..t, tile (64,0) does chunks t+1..2t+1.
    kT/qT are duplicated into both SBUF partition halves (SBUF->SBUF DMA).
  - Gathered Q^T stays source-major [128, 2, 16, 128]; score matmuls use two
    N=256 strided-AP pieces so no interleave scatter-DMA is ever done.
  - exp runs on the scalar engine over 3-bank PSUM score sets (N=1536,
    24 uniform sets of 3 chunks), scale=1/8 fused, bf16 out.
  - V is augmented with a ones column so the AV matmul also produces the
    softmax denominator (row 64 of the [65, 512] accumulator).
"""

import sys

sys.path.insert(0, "/opt/trn_rl_repo")

import numpy as np
import ml_dtypes

B, S, D, H = 4, 4096, 1024, 64
RPC = S // 2            # rows (keys/queries) owned per core
QB = 512                # query block width
NQB = S // QB           # 8 query blocks
NKC = RPC // 128        # 16 local key chunks
NSG = RPC // QB         # 4 own-row groups for projection
BF16 = ml_dtypes.bfloat16
PAIRS = [[0, 1], [2, 3], [4, 5], [6, 7]]

_CACHE = {}


def _build():
    import concourse.bass as bass
    import concourse.mybir as mybir
    from concourse import bacc, tile
    from concourse.bass import ts

    f32 = mybir.dt.float32
    bf16 = mybir.dt.bfloat16
    Alu = mybir.AluOpType
    Act = mybir.ActivationFunctionType

    nc = bacc.Bacc(None, target_bir_lowering=False)

    x_ext = nc.declare_dram_parameter("x", [D, RPC], bf16, isOutput=False)
    wkq_ext = nc.declare_dram_parameter("wkq", [D, 128], bf16, isOutput=False)
    wv_ext = nc.declare_dram_parameter("wv", [D, H], bf16, isOutput=False)
    mask_ext = nc.declare_dram_parameter("mask", [2, 128, QB], bf16, isOutput=False)
    out_ext = nc.declare_dram_parameter("out", [H + 1, S], f32, isOutput=True)

    with tile.TileContext(nc) as tc:
        with (
            tc.tile_pool(name="persist", bufs=1) as persist,
            tc.tile_pool(name="dram", bufs=1, space="DRAM") as dram,
        ):
            # --- persistent SBUF tensors ---
            wkq_sb = persist.tile([128, 8, 128], bf16, tag="wkq")
            wv_sb = persist.tile([128, 8, H], bf16, tag="wv")
            mask_sb = persist.tile([128, 2, QB], bf16, tag="mask")
            # kT2: [0:64] and [64:128] both hold all 16 local chunks
            kT2 = persist.tile([128, NKC, 128], bf16, tag="kT2")
            # gathered q, source-major; both partition halves hold a copy
            qT2g = persist.tile([128, 2, NKC, 128], bf16, tag="qT2g")
            v_all = persist.tile([128, NKC, H + 2], bf16, tag="v_all")
            qtmp = persist.tile([128, RPC], bf16, tag="qtmp")
            zjunk = persist.tile([128, 8], f32, tag="zjunk")
            ejunk = persist.tile([128, 8], bf16, tag="ejunk")

            # preload the exp activation table set early (it costs ~2.7us)
            nc.vector.memset(zjunk[:], 0.0)
            nc.scalar.activation(ejunk[:], zjunk[:], Act.Exp)
            nc.vector.memset(v_all[:, :, H], 1.0)

            for dc in range(8):
                nc.sync.dma_start(out=wkq_sb[:, dc, :], in_=wkq_ext[ts(dc, 128), :])
                nc.sync.dma_start(out=wv_sb[:, dc, :], in_=wv_ext[ts(dc, 128), :])
            nc.sync.dma_start(out=mask_sb[:, 0, :], in_=mask_ext[0])
            nc.sync.dma_start(out=mask_sb[:, 1, :], in_=mask_ext[1])

            # --- phase 1: linear x^T loads + projections over own rows ---
            with (
                tc.tile_pool(name="xt", bufs=2) as xt_pool,
                tc.tile_pool(name="pj", bufs=2, space="PSUM") as pj_pool,
                tc.tile_pool(name="pv", bufs=2, space="PSUM") as pv_pool,
            ):
                for sg in range(NSG):
                    xT = xt_pool.tile([128, 8, QB], bf16, tag="xT")
                    for dc in range(8):
                        nc.sync.dma_start(
                            out=xT[:, dc, :],
                            in_=x_ext[ts(dc, 128), ts(sg, QB)],
                        )
                    # k|q packed projection: k -> psum[0:64], q -> psum[64:128]
                    kq_ps = pj_pool.tile([128, QB], f32, tag="kq")
                    for dc in range(8):
                        nc.tensor.matmul(
                            kq_ps[:],
                            lhsT=wkq_sb[:, dc, :],
                            rhs=xT[:, dc, :],
                            start=(dc == 0),
                            stop=(dc == 7),
                        )
                    for kc in range(4):
                        nc.any.tensor_copy(
                            kT2[0:64, 4 * sg + kc, :], kq_ps[0:64, ts(kc, 128)]
                        )
                    nc.any.tensor_copy(qtmp[64:128, ts(sg, QB)], kq_ps[64:128, :])
                    # natural-layout V: x^T chunk stationary, Wv chunk moving
                    for kc in range(4):
                        v_ps = pv_pool.tile([128, H], f32, tag="v")
                        for dc in range(8):
                            nc.tensor.matmul(
                                v_ps[:],
                                lhsT=xT[:, dc, ts(kc, 128)],
                                rhs=wv_sb[:, dc, :],
                                start=(dc == 0),
                                stop=(dc == 7),
                            )
                        nc.any.tensor_copy(v_all[:, 4 * sg + kc, 0:H], v_ps[:])

            # --- phase 2: pair AllGather of Q^T; duplicate kT/qT halves ---
            q_bounce = dram.tile([64, RPC], bf16, tag="q_bounce")
            q_gath = dram.tile([2, 64, RPC], bf16, tag="q_gath")
            nc.sync.dma_start(out=q_bounce[:], in_=qtmp[64:128, :])
            nc.gpsimd.collective_compute(
                "AllGather",
                Alu.bypass,
                replica_groups=PAIRS,
                ins=[q_bounce.opt()],
                outs=[q_gath.opt()],
            )
            # kT high-half duplicate (SBUF->SBUF, partition shift)
            nc.sync.dma_start(out=kT2[64:128, :, :], in_=kT2[0:64, :, :])
            # gathered q into both partition halves, source-major layout
            for src in range(2):
                nc.sync.dma_start(out=qT2g[0:64, src, :, :], in_=q_gath[src])
                nc.sync.dma_start(out=qT2g[64:128, src, :, :], in_=q_gath[src])

            # --- phase 3: attention ---
            with (
                tc.tile_pool(name="st", bufs=2, space="PSUM") as st_pool,
                tc.tile_pool(name="av", bufs=2, space="PSUM") as av_pool,
                tc.tile_pool(name="p", bufs=10) as p_pool,
                tc.tile_pool(name="o", bufs=3) as o_pool,
            ):
                gamma = 0
                cur_st = None
                cur_p = None
                pend_masks = []
                chunk_p = {}

                def emit_av(t):
                    E = 2 * (t + 1)
                    av = av_pool.tile([H + 1, QB], f32, tag="av")
                    for cid in range(E):
                        pt, pos = chunk_p[t][cid]
                        nc.tensor.matmul(
                            av[:],
                            lhsT=v_all[:, cid, 0 : H + 1],
                            rhs=pt[:, pos, :],
                            start=(cid == 0),
                            stop=(cid == E - 1),
                        )
                    o = o_pool.tile([H + 1, QB], f32, tag="o")
                    nc.vector.tensor_copy(o[:], av[:])
                    nc.sync.dma_start(out=out_ext[:, ts(t, QB)], in_=o[:])

                for t in range(NQB):
                    E = 2 * (t + 1)
                    chunk_p[t] = [None] * E
                    for s in range(t + 1):
                        for h in (0, 1):
                            cid = s if h == 0 else t + 1 + s
                            if cur_st is None:
                                cur_st = st_pool.tile([128, 3, QB], f32, tag="st")
                                cur_p = p_pool.tile([128, 3, QB], bf16, tag="p")
                            pos = gamma % 3
                            for pc in (0, 1):
                                nc.tensor.matmul(
                                    cur_st[:, pos, ts(pc, 256)],
                                    lhsT=kT2[64 * h : 64 * h + 64, cid, :],
                                    rhs=qT2g[64 * h : 64 * h + 64, :, 2 * t + pc, :],
                                    start=True,
                                    stop=True,
                                    tile_position=(64 * h, 0),
                                )
                            chunk_p[t][cid] = (cur_p, pos)
                            if cid >= E - 2:
                                pend_masks.append((cur_p, pos, cid - (E - 2)))
                            gamma += 1
                            if gamma % 3 == 0:
                                nc.scalar.activation(
                                    cur_p[:], cur_st[:], Act.Exp, scale=0.125
                                )
                                for pt, pp, j in pend_masks:
                                    nc.vector.tensor_tensor(
                                        pt[:, pp, :],
                                        pt[:, pp, :],
                                        mask_sb[:, j, :],
                                        Alu.mult,
                                    )
                                pend_masks = []
                                cur_st = None
                                cur_p = None
                    if t >= 1:
                        emit_av(t - 1)
                emit_av(NQB - 1)

    nc.finalize()
    return nc


def _make_masks(g: int) -> np.ndarray:
    # mask[j][kk, qq] = 1 if query (512t + qq) >= key 128*(4t + 2j + g) + kk
    m = np.zeros((2, 128, QB), dtype=np.float32)
    for j in range(2):
        dk = 128 * (2 * j + g) + np.arange(128)[:, None]
        dq = np.arange(QB)[None, :]
        m[j] = (dq >= dk).astype(np.float32)
    return m.astype(BF16)


def _shard_inputs(input, Wq, Wk, Wv):
    x = np.asarray(input)
    wkq = np.concatenate([Wk, Wq], axis=1).astype(BF16)
    wv = np.ascontiguousarray(Wv).astype(BF16)
    masks = [_make_masks(0), _make_masks(1)]
    in_maps = []
    for c in range(8):
        b, g = c // 2, c % 2
        xs = x[b].reshape(S // 128, 128, D)[g::2].reshape(RPC, D)
        xT = np.ascontiguousarray(xs.T).astype(BF16)
        in_maps.append({"x": xT, "wkq": wkq, "wv": wv, "mask": masks[g]})
    return in_maps


def _unshard(results):
    out = np.empty((B, S, H), dtype=np.float32)
    for b in range(B):
        merged = results[2 * b]["out"] + results[2 * b + 1]["out"]
        out[b] = (merged[:H] / merged[H : H + 1]).T
    return out


def _run(inputs, trace=False):
    from concourse.bass_utils import run_bass_kernel_spmd

    if "nc" not in _CACHE:
        _CACHE["nc"] = _build()
    nc = _CACHE["nc"]
    in_maps = _shard_inputs(**inputs)
    res = run_bass_kernel_spmd(nc, in_maps, core_ids=list(range(8)), trace=trace)
    out = _unshard(res.results)
    return out, res


def kernel(**inputs) -> np.ndarray:
    out, _ = _run(inputs, trace=False)
    return out
